# revision 22
# baseline (speedup 1.0000x reference)
"""Bahdanau additive attention on 8 Trainium2 NeuronCores.

Shapes (hardcoded from the problem spec):
  encoder_out [B=4, Te=512, De=512], decoder_out [B=4, Td=256, Dd=512]
  W1 [512,128], W2 [512,128], V [128,1]; U=128.
Outputs: context [4,256,512], attn_weights [4,256,512].

Sharding: core c handles batch b=c//2, decoder rows (c%2)*128..+128.

Per-core pipeline (U=128 on SBUF partitions for phase 1):
  encT    = PE-transpose(enc)                  [De,Te]   (per-te-chunk pipelined)
  enc_pT  = W1^T @ encT (+b1)                  [U,Te]    (fp32r single-pass)
  dec_pT  = W2^T @ decT (+b2)                  [U,Td]
  per td: pre = enc_pT + dec_pT[:,td]          (DVE/GPSIMD tensor_scalar_add)
          h   = tanh(pre) -> bf16              (ACT, batched over SUB tds)
  scores land in NATURAL [td, te] layout via accumulating matmuls with a
  sliding-window stationary (v in column j):
          score[32-group] += Zwin_j^T @ h_td_j   (PE, bf16 moving N=512)
  softmax row-wise: exp+accum_out (ACT) -> rinv (DVE) -> attn = esc*rinv
  ctx per 32-td group: PE-transpose attn quarter -> ctx = attnT^T @ enc (fp32r)
"""

import numpy as np

B, TE, TD, DE, U = 4, 512, 256, 512, 128
N_CORES = 8
ROWS = 128  # decoder rows per core
SUB = 8  # tds per tanh batch
GRP = 32  # tds per score accumulation group / ctx quarter

_CACHE = {}


def _build_program():
    from contextlib import ExitStack

    import concourse.bacc as bacc
    import concourse.tile as tile
    from concourse import mybir
    from concourse.masks import make_identity

    f32 = mybir.dt.float32
    f32r = mybir.dt.float32r
    bf16 = mybir.dt.bfloat16
    AF = mybir.ActivationFunctionType

    nc = bacc.Bacc("TRN2", target_bir_lowering=False, debug=False)

    enc_d = nc.dram_tensor("enc", [TE, DE], f32, kind="ExternalInput")
    dec_d = nc.dram_tensor("dec", [ROWS, DE], f32, kind="ExternalInput")
    w1r_d = nc.dram_tensor("w1r", [DE, U], f32r, kind="ExternalInput")
    w2r_d = nc.dram_tensor("w2r", [DE, U], f32r, kind="ExternalInput")
    encr_d = nc.dram_tensor("encr", [TE, DE], f32r, kind="ExternalInput")
    v_d = nc.dram_tensor("v", [U, 1], f32, kind="ExternalInput")
    w1b_d = nc.dram_tensor("w1b", [U], f32, kind="ExternalInput")
    w2b_d = nc.dram_tensor("w2b", [U], f32, kind="ExternalInput")
    ctx_d = nc.dram_tensor("ctx", [ROWS, DE], f32, kind="ExternalOutput")
    attn_d = nc.dram_tensor("attn", [ROWS, TE], f32, kind="ExternalOutput")

    NT = TE // 128  # te chunks
    ND = DE // 128  # de chunks

    with tile.TileContext(nc) as tc, ExitStack() as ctx:
        const = ctx.enter_context(tc.tile_pool(name="const", bufs=1))
        work = ctx.enter_context(tc.tile_pool(name="work", bufs=3))
        att = ctx.enter_context(tc.tile_pool(name="att", bufs=2))
        ps_t = ctx.enter_context(tc.tile_pool(name="ps_t", bufs=2, space="PSUM"))
        ps_p = ctx.enter_context(tc.tile_pool(name="ps_p", bufs=1, space="PSUM"))
        ps_v = ctx.enter_context(tc.tile_pool(name="ps_v", bufs=2, space="PSUM"))
        ps_c = ctx.enter_context(tc.tile_pool(name="ps_c", bufs=2, space="PSUM"))

        ident = const.tile([128, 128], f32, tag="ident")
        make_identity(nc, ident)

        # --- input DMAs, split across the two HWDGE rings ---
        enc_sb = [
            const.tile([128, DE], f32, tag=f"enc_{t}", name=f"enc_{t}")
            for t in range(NT)
        ]
        dec_sb = const.tile([ROWS, DE], f32, tag="dec")
        # ring1 (SP): enc0, w2, w1, enc2 ...; ring2 (ACT): dec, enc1, enc3 ...
        nc.sync.dma_start(out=enc_sb[0], in_=enc_d[0:128, :])
        nc.scalar.dma_start(out=dec_sb, in_=dec_d[:, :])
        w2_r = const.tile([128, ND, U], f32r, tag="w2r")
        nc.sync.dma_start(out=w2_r, in_=w2r_d.rearrange("(k p) u -> p k u", p=128))
        w1_r = const.tile([128, ND, U], f32r, tag="w1r")
        nc.sync.dma_start(out=w1_r, in_=w1r_d.rearrange("(k p) u -> p k u", p=128))
        nc.scalar.dma_start(out=enc_sb[1], in_=enc_d[128:256, :])
        nc.sync.dma_start(out=enc_sb[2], in_=enc_d[256:384, :])
        nc.scalar.dma_start(out=enc_sb[3], in_=enc_d[384:512, :])
        v_sb = const.tile([U, 1], f32, tag="v")
        nc.sync.dma_start(out=v_sb, in_=v_d[:, :])
        w1b_sb = const.tile([U, 1], f32, tag="w1b")
        nc.sync.dma_start(out=w1b_sb, in_=w1b_d[:, None])
        w2b_sb = const.tile([U, 1], f32, tag="w2b")
        nc.scalar.dma_start(out=w2b_sb, in_=w2b_d[:, None])
        # enc f32r copies for the ctx matmul rhs: DMA'd late (needed ~60us in)
        enc_r = []
        for t in range(NT):
            er = const.tile([128, DE], f32r, tag=f"encr_{t}", name=f"encr_{t}")
            eng = nc.sync if t % 2 == 0 else nc.scalar
            eng.dma_start(out=er, in_=encr_d[t * 128 : (t + 1) * 128, :])
            enc_r.append(er)

        # sliding-window stationary: Zwin[:, (GRP-1)-j : (2*GRP-1)-j] puts
        # v (bf16) in column j of a [U, GRP] stationary, zeros elsewhere
        zwin = const.tile([U, 2 * GRP - 1], bf16, tag="zwin")
        nc.vector.memset(zwin, 0.0)
        nc.vector.tensor_copy(zwin[:, GRP - 1 : GRP], v_sb)

        # --- setup interleaved with early (te-chunked) tanh for tds 0..7 ---
        # encT stored d-major: encT_d [de-part, te] f32r
        encT = [
            const.tile([128, TE], f32r, tag=f"encT_{d}", name=f"encT_{d}")
            for d in range(ND)
        ]
        ep = ps_p.tile([U, TE], f32, tag="ep", name="ep")
        enc_pT = const.tile([U, TE], bf16, tag="enc_pT")
        attn_sb = const.tile([ROWS, TE], f32, tag="attn_sb")
        vout0 = ps_v.tile([64, TE], f32, tag="vout", name="vout0")
        pre_r = work.tile([128, 8, TE], bf16, tag="pre_r", bufs=1)
        th_r = work.tile([128, 8, TE], bf16, tag="th_r", bufs=1)

        dec_pT = None

        def enc_chunk(t):
            tp = ps_t.tile([128, ND, 128], f32, tag="tp", name=f"tp_e{t}")
            for d in range(ND):
                nc.tensor.transpose(
                    tp[:, d, :], enc_sb[t][:, d * 128 : (d + 1) * 128], ident
                )
            for d in range(ND):
                nc.vector.tensor_copy(encT[d][:, t * 128 : (t + 1) * 128], tp[:, d, :])
            sl = slice(t * 128, (t + 1) * 128)
            for d in range(ND):
                nc.tensor.matmul(
                    ep[:, sl],
                    w1_r[:, d, :],
                    encT[d][:, sl],
                    start=(d == 0),
                    stop=(d == ND - 1),
                )
            nc.vector.tensor_scalar_add(enc_pT[:, sl], ep[:, sl], w1b_sb)

        def ramp_chunk(t):
            sl = slice(t * 128, (t + 1) * 128)
            for j in range(8):
                nc.vector.tensor_scalar_add(
                    pre_r[:, j, sl], enc_pT[:, sl], dec_pT[:, j : j + 1]
                )
            nc.scalar.activation(th_r[:, :, sl], pre_r[:, :, sl], AF.Tanh)

        enc_chunk(0)

        # dec: transpose + proj + bias, right after chunk 0
        tpd = ps_t.tile([128, ND, 128], f32, tag="tp", name="tp_d")
        for d in range(ND):
            nc.tensor.transpose(tpd[:, d, :], dec_sb[:, d * 128 : (d + 1) * 128], ident)
        decT = const.tile([128, ND, 128], f32r, tag="decT")
        nc.vector.tensor_copy(decT, tpd)
        dp = ps_p.tile([U, ROWS], f32, tag="dp", name="dp")
        for d in range(ND):
            nc.tensor.matmul(
                dp,
                w2_r[:, d, :],
                decT[:, d, :],
                start=(d == 0),
                stop=(d == ND - 1),
            )
        dec_pT = const.tile([U, ROWS], f32, tag="dec_pT")
        nc.vector.tensor_scalar_add(dec_pT, dp, w2b_sb)

        ramp_chunk(0)
        for t in range(1, NT):
            enc_chunk(t)
            ramp_chunk(t)
        for j in range(8):
            nc.tensor.matmul(
                vout0[0:GRP, :],
                zwin[:, (GRP - 1) - j : (2 * GRP - 1) - j],
                th_r[:, j, :],
                start=(j == 0),
                stop=False,
            )

        # --- adds + tanh + score accumulation (tds 8..127) ---
        n_half = ROWS // 64
        ramp_sched = [(8, 12, 0), (20, 12, 0), (32, 12, 0), (44, 12, 0), (56, 8, 0)]
        last_sched = [(0, 12, 0), (12, 12, 0), (24, 12, 0), (36, 12, 0),
                      (48, 8, 0), (56, 4, 0), (60, 4, 0)]
        vouts = []
        for half in range(n_half):
            if half == 0:
                vout = vout0
            else:
                vout = ps_v.tile([64, TE], f32, tag="vout", name=f"vout{half}")
            vouts.append(vout)
            sched = ramp_sched if half == 0 else last_sched
            for s0, ns, chunked in sched:
                pre = work.tile([128, 12, TE], bf16, tag="pre", name="pre")
                th = work.tile([128, 12, TE], bf16, tag="th", name="th")
                for j in range(ns):
                    td = half * 64 + s0 + j
                    nc.vector.tensor_scalar_add(
                        pre[:, j, :], enc_pT, dec_pT[:, td : td + 1]
                    )
                nc.scalar.activation(th[:, :ns, :], pre[:, :ns, :], AF.Tanh)
                for j in range(ns):
                    r = s0 + j  # row within this 64-row vout tile
                    g = r // GRP  # 0 or 1
                    jj = r % GRP  # position in group -> stationary column
                    nc.tensor.matmul(
                        vout[g * GRP : (g + 1) * GRP, :],
                        zwin[:, (GRP - 1) - jj : (2 * GRP - 1) - jj],
                        th[:, j, :],
                        start=(jj == 0),
                        stop=(jj == GRP - 1),
                    )

            # softmax rows (no max subtraction: |score| <= |v|_1 ~ 9)
            r0 = half * 64
            esc = att.tile([64, TE], f32, tag="esc", name="esc")
            esum = work.tile([64, 1], f32, tag="esum", name="esum")
            nc.scalar.activation(esc, vout, AF.Exp, accum_out=esum)
            rinv = work.tile([64, 1], f32, tag="rinv", name="rinv")
            nc.vector.reciprocal(rinv, esum)
            nc.vector.tensor_scalar_mul(attn_sb[r0 : r0 + 64, :], esc, rinv)
            nc.sync.dma_start(
                out=attn_d[r0 : r0 + 64, :], in_=attn_sb[r0 : r0 + 64, :]
            )
            # context from unnormalized esc; normalization fused in the copy
            at = ps_t.tile([128, NT, 64], f32, tag="tp", name=f"at{half}")
            for t in range(NT):
                nc.tensor.transpose(
                    at[:, t, :],
                    esc[:, t * 128 : (t + 1) * 128],
                    ident[0:64, 0:64],
                )
            escT = att.tile([128, NT, 64], f32r, tag="escT", name="escT")
            nc.vector.tensor_copy(escT, at)
            ctx_ps = ps_c.tile([64, DE], f32, tag="ctx", name="ctx_ps")
            for t in range(NT):
                nc.tensor.matmul(
                    ctx_ps,
                    escT[:, t, :],
                    enc_r[t],
                    start=(t == 0),
                    stop=(t == NT - 1),
                )
            ctx_sb = att.tile([64, DE], f32, tag="ctx_sb", name="ctx_sb")
            nc.vector.tensor_scalar_mul(ctx_sb, ctx_ps, rinv)
            nc.sync.dma_start(out=ctx_d[r0 : r0 + 64, :], in_=ctx_sb)

    nc.compile()
    return nc


def _get_nc():
    if "nc" not in _CACHE:
        _CACHE["nc"] = _build_program()
    return _CACHE["nc"]


def _install_ntff_hook():
    """The agent image's antenv lacks axon_hooks; synthesize it so
    run_bass_kernel_spmd(trace=True) can reach the boot shim's
    ctypes-based NTFF profiler."""
    import sys
    import types

    if "antenv.axon_hooks" not in sys.modules:
        mod = types.ModuleType("antenv.axon_hooks")
        mod._hook = None
        mod.set_axon_ntff_profile_hook = lambda h: setattr(mod, "_hook", h)
        mod.get_axon_ntff_profile_hook = lambda: mod._hook
        sys.modules["antenv.axon_hooks"] = mod
        try:
            from trn_agent_boot.trn_boot import _ntff_profile_via_ctypes

            mod._hook = _ntff_profile_via_ctypes("/opt/axon/libaxon_pjrt.so")
        except Exception as e:
            print(f"ntff hook install failed: {e}")
    import concourse.bass_utils as bu

    bu.upload_artifacts = lambda tmpdir: "local://" + str(tmpdir)


def run(inputs, trace=False):
    from concourse.bass_utils import run_bass_kernel_spmd

    if trace:
        _install_ntff_hook()

    nc = _get_nc()
    enc = np.asarray(inputs["encoder_out"], dtype=np.float32)
    dec = np.asarray(inputs["decoder_out"], dtype=np.float32)
    w1 = np.ascontiguousarray(inputs["W1_w"], dtype=np.float32)
    w2 = np.ascontiguousarray(inputs["W2_w"], dtype=np.float32)
    v = np.ascontiguousarray(inputs["V_w"], dtype=np.float32)
    w1b = np.ascontiguousarray(inputs["W1_b"], dtype=np.float32)
    w2b = np.ascontiguousarray(inputs["W2_b"], dtype=np.float32)

    in_maps = []
    for c in range(N_CORES):
        b, h = c // 2, c % 2
        in_maps.append(
            {
                "enc": np.ascontiguousarray(enc[b]),
                "dec": np.ascontiguousarray(dec[b, h * ROWS : (h + 1) * ROWS]),
                "w1r": w1,
                "w2r": w2,
                "encr": np.ascontiguousarray(enc[b]),
                "v": v,
                "w1b": w1b,
                "w2b": w2b,
            }
        )

    res = run_bass_kernel_spmd(nc, in_maps, list(range(N_CORES)), trace=trace)

    context = np.empty((B, TD, DE), np.float32)
    attn = np.empty((B, TD, TE), np.float32)
    for c in range(N_CORES):
        b, h = c // 2, c % 2
        context[b, h * ROWS : (h + 1) * ROWS] = res.results[c]["ctx"]
        attn[b, h * ROWS : (h + 1) * ROWS] = res.results[c]["attn"]
    return (context, attn), res


def kernel(**inputs):
    (context, attn), _ = run(inputs)
    return context, attn


# revision 23
# speedup vs baseline: 1.0032x; 1.0032x over previous
"""Bahdanau additive attention on 8 Trainium2 NeuronCores.

Shapes (hardcoded from the problem spec):
  encoder_out [B=4, Te=512, De=512], decoder_out [B=4, Td=256, Dd=512]
  W1 [512,128], W2 [512,128], V [128,1]; U=128.
Outputs: context [4,256,512], attn_weights [4,256,512].

Sharding: core c handles batch b=c//2, decoder rows (c%2)*128..+128.

Per-core pipeline (U=128 on SBUF partitions for phase 1):
  encT    = PE-transpose(enc)                  [De,Te]   (per-te-chunk pipelined)
  enc_pT  = W1^T @ encT (+b1)                  [U,Te]    (fp32r single-pass)
  dec_pT  = W2^T @ decT (+b2)                  [U,Td]
  per td: pre = enc_pT + dec_pT[:,td]          (DVE/GPSIMD tensor_scalar_add)
          h   = tanh(pre) -> bf16              (ACT, batched over SUB tds)
  scores land in NATURAL [td, te] layout via accumulating matmuls with a
  sliding-window stationary (v in column j):
          score[32-group] += Zwin_j^T @ h_td_j   (PE, bf16 moving N=512)
  softmax row-wise: exp+accum_out (ACT) -> rinv (DVE) -> attn = esc*rinv
  ctx per 32-td group: PE-transpose attn quarter -> ctx = attnT^T @ enc (fp32r)
"""

import numpy as np

B, TE, TD, DE, U = 4, 512, 256, 512, 128
N_CORES = 8
ROWS = 128  # decoder rows per core
SUB = 8  # tds per tanh batch
GRP = 32  # tds per score accumulation group / ctx quarter

_CACHE = {}


def _build_program():
    from contextlib import ExitStack

    import concourse.bacc as bacc
    import concourse.tile as tile
    from concourse import mybir
    from concourse.masks import make_identity

    f32 = mybir.dt.float32
    f32r = mybir.dt.float32r
    bf16 = mybir.dt.bfloat16
    AF = mybir.ActivationFunctionType

    nc = bacc.Bacc("TRN2", target_bir_lowering=False, debug=False)

    enc_d = nc.dram_tensor("enc", [TE, DE], f32, kind="ExternalInput")
    dec_d = nc.dram_tensor("dec", [ROWS, DE], f32, kind="ExternalInput")
    w1r_d = nc.dram_tensor("w1r", [DE, U], f32r, kind="ExternalInput")
    w2r_d = nc.dram_tensor("w2r", [DE, U], f32r, kind="ExternalInput")
    encr_d = nc.dram_tensor("encr", [TE, DE], f32r, kind="ExternalInput")
    v_d = nc.dram_tensor("v", [U, 1], f32, kind="ExternalInput")
    w1b_d = nc.dram_tensor("w1b", [U], f32, kind="ExternalInput")
    w2b_d = nc.dram_tensor("w2b", [U], f32, kind="ExternalInput")
    ctx_d = nc.dram_tensor("ctx", [ROWS, DE], f32, kind="ExternalOutput")
    attn_d = nc.dram_tensor("attn", [ROWS, TE], f32, kind="ExternalOutput")

    NT = TE // 128  # te chunks
    ND = DE // 128  # de chunks

    with tile.TileContext(nc) as tc, ExitStack() as ctx:
        const = ctx.enter_context(tc.tile_pool(name="const", bufs=1))
        work = ctx.enter_context(tc.tile_pool(name="work", bufs=3))
        att = ctx.enter_context(tc.tile_pool(name="att", bufs=2))
        ps_t = ctx.enter_context(tc.tile_pool(name="ps_t", bufs=2, space="PSUM"))
        ps_p = ctx.enter_context(tc.tile_pool(name="ps_p", bufs=1, space="PSUM"))
        ps_v = ctx.enter_context(tc.tile_pool(name="ps_v", bufs=2, space="PSUM"))
        ps_c = ctx.enter_context(tc.tile_pool(name="ps_c", bufs=2, space="PSUM"))

        ident = const.tile([128, 128], f32, tag="ident")
        make_identity(nc, ident)

        # --- input DMAs, split across the two HWDGE rings ---
        enc_sb = [
            const.tile([128, DE], f32, tag=f"enc_{t}", name=f"enc_{t}")
            for t in range(NT)
        ]
        dec_sb = const.tile([ROWS, DE], f32, tag="dec")
        # ring1 (SP): enc0, w2, w1, enc2 ...; ring2 (ACT): dec, enc1, enc3 ...
        nc.sync.dma_start(out=enc_sb[0], in_=enc_d[0:128, :])
        nc.scalar.dma_start(out=dec_sb, in_=dec_d[:, :])
        w2_r = const.tile([128, ND, U], f32r, tag="w2r")
        nc.sync.dma_start(out=w2_r, in_=w2r_d.rearrange("(k p) u -> p k u", p=128))
        w1_r = const.tile([128, ND, U], f32r, tag="w1r")
        nc.sync.dma_start(out=w1_r, in_=w1r_d.rearrange("(k p) u -> p k u", p=128))
        nc.scalar.dma_start(out=enc_sb[1], in_=enc_d[128:256, :])
        nc.sync.dma_start(out=enc_sb[2], in_=enc_d[256:384, :])
        nc.scalar.dma_start(out=enc_sb[3], in_=enc_d[384:512, :])
        v_sb = const.tile([U, 1], f32, tag="v")
        nc.sync.dma_start(out=v_sb, in_=v_d[:, :])
        w1b_sb = const.tile([U, 1], f32, tag="w1b")
        nc.sync.dma_start(out=w1b_sb, in_=w1b_d[:, None])
        w2b_sb = const.tile([U, 1], f32, tag="w2b")
        nc.scalar.dma_start(out=w2b_sb, in_=w2b_d[:, None])
        # enc f32r copies for the ctx matmul rhs: DMA'd late (needed ~60us in)
        enc_r = []
        for t in range(NT):
            er = const.tile([128, DE], f32r, tag=f"encr_{t}", name=f"encr_{t}")
            eng = nc.sync if t % 2 == 0 else nc.scalar
            eng.dma_start(out=er, in_=encr_d[t * 128 : (t + 1) * 128, :])
            enc_r.append(er)

        # sliding-window stationary: Zwin[:, (GRP-1)-j : (2*GRP-1)-j] puts
        # v (bf16) in column j of a [U, GRP] stationary, zeros elsewhere
        zwin = const.tile([U, 2 * GRP - 1], bf16, tag="zwin")
        nc.vector.memset(zwin, 0.0)
        nc.vector.tensor_copy(zwin[:, GRP - 1 : GRP], v_sb)

        # --- setup interleaved with early (te-chunked) tanh for tds 0..7 ---
        # encT stored d-major: encT_d [de-part, te] f32r
        encT = [
            const.tile([128, TE], f32r, tag=f"encT_{d}", name=f"encT_{d}")
            for d in range(ND)
        ]
        ep = ps_p.tile([U, TE], f32, tag="ep", name="ep")
        enc_pT = const.tile([U, TE], bf16, tag="enc_pT")
        attn_sb = const.tile([ROWS, TE], f32, tag="attn_sb")
        vout0 = ps_v.tile([64, TE], f32, tag="vout", name="vout0")
        pre_r = work.tile([128, 8, TE], bf16, tag="pre_r", bufs=1)
        th_r = work.tile([128, 8, TE], bf16, tag="th_r", bufs=1)

        dec_pT = None

        def enc_chunk(t):
            tp = ps_t.tile([128, ND, 128], f32, tag="tp", name=f"tp_e{t}")
            for d in range(ND):
                nc.tensor.transpose(
                    tp[:, d, :], enc_sb[t][:, d * 128 : (d + 1) * 128], ident
                )
            for d in range(ND):
                nc.vector.tensor_copy(encT[d][:, t * 128 : (t + 1) * 128], tp[:, d, :])
            sl = slice(t * 128, (t + 1) * 128)
            for d in range(ND):
                nc.tensor.matmul(
                    ep[:, sl],
                    w1_r[:, d, :],
                    encT[d][:, sl],
                    start=(d == 0),
                    stop=(d == ND - 1),
                )
            nc.vector.tensor_scalar_add(enc_pT[:, sl], ep[:, sl], w1b_sb)

        def ramp_chunk(t):
            sl = slice(t * 128, (t + 1) * 128)
            for j in range(8):
                nc.vector.tensor_scalar_add(
                    pre_r[:, j, sl], enc_pT[:, sl], dec_pT[:, j : j + 1]
                )
            nc.scalar.activation(th_r[:, :, sl], pre_r[:, :, sl], AF.Tanh)

        enc_chunk(0)

        # dec: transpose + proj + bias, right after chunk 0
        tpd = ps_t.tile([128, ND, 128], f32, tag="tp", name="tp_d")
        for d in range(ND):
            nc.tensor.transpose(tpd[:, d, :], dec_sb[:, d * 128 : (d + 1) * 128], ident)
        decT = const.tile([128, ND, 128], f32r, tag="decT")
        nc.vector.tensor_copy(decT, tpd)
        dp = ps_p.tile([U, ROWS], f32, tag="dp", name="dp")
        for d in range(ND):
            nc.tensor.matmul(
                dp,
                w2_r[:, d, :],
                decT[:, d, :],
                start=(d == 0),
                stop=(d == ND - 1),
            )
        dec_pT = const.tile([U, ROWS], f32, tag="dec_pT")
        nc.vector.tensor_scalar_add(dec_pT, dp, w2b_sb)

        ramp_chunk(0)
        for t in range(1, NT):
            enc_chunk(t)
            ramp_chunk(t)
        for j in range(8):
            nc.tensor.matmul(
                vout0[0:GRP, :],
                zwin[:, (GRP - 1) - j : (2 * GRP - 1) - j],
                th_r[:, j, :],
                start=(j == 0),
                stop=False,
            )

        # --- adds + tanh + score accumulation (tds 8..127), flat schedule ---
        # epilogue (softmax+ctx) for a half is emitted AFTER the next half's
        # first two sub-batches so DVE's in-order stream keeps feeding ACT.
        vout1 = ps_v.tile([64, TE], f32, tag="vout", name="vout1")
        vouts = [vout0, vout1]

        def sub_batch(half, s0, ns):
            vout = vouts[half]
            pre = work.tile([128, SUB, TE], bf16, tag="pre", name="pre")
            th = work.tile([128, SUB, TE], bf16, tag="th", name="th")
            for j in range(ns):
                td = half * 64 + s0 + j
                nc.vector.tensor_scalar_add(
                    pre[:, j, :], enc_pT, dec_pT[:, td : td + 1]
                )
            nc.scalar.activation(th[:, :ns, :], pre[:, :ns, :], AF.Tanh)
            for j in range(ns):
                r = s0 + j
                g = r // GRP
                jj = r % GRP
                nc.tensor.matmul(
                    vout[g * GRP : (g + 1) * GRP, :],
                    zwin[:, (GRP - 1) - jj : (2 * GRP - 1) - jj],
                    th[:, j, :],
                    start=(jj == 0),
                    stop=(jj == GRP - 1),
                )

        def epilogue(half):
            # softmax rows (no max subtraction: |score| <= |v|_1 ~ 9)
            r0 = half * 64
            vout = vouts[half]
            esc = att.tile([64, TE], f32, tag="esc", name="esc")
            esum = work.tile([64, 1], f32, tag="esum", name="esum")
            nc.scalar.activation(esc, vout, AF.Exp, accum_out=esum)
            rinv = work.tile([64, 1], f32, tag="rinv", name="rinv")
            nc.vector.reciprocal(rinv, esum)
            nc.vector.tensor_scalar_mul(attn_sb[r0 : r0 + 64, :], esc, rinv)
            nc.sync.dma_start(
                out=attn_d[r0 : r0 + 64, :], in_=attn_sb[r0 : r0 + 64, :]
            )
            # context from unnormalized esc; normalization fused in the copy
            at = ps_t.tile([128, NT, 64], f32, tag="tp", name=f"at{half}")
            for t in range(NT):
                nc.tensor.transpose(
                    at[:, t, :],
                    esc[:, t * 128 : (t + 1) * 128],
                    ident[0:64, 0:64],
                )
            escT = att.tile([128, NT, 64], f32r, tag="escT", name="escT")
            nc.vector.tensor_copy(escT, at)
            ctx_ps = ps_c.tile([64, DE], f32, tag="ctx", name="ctx_ps")
            for t in range(NT):
                nc.tensor.matmul(
                    ctx_ps,
                    escT[:, t, :],
                    enc_r[t],
                    start=(t == 0),
                    stop=(t == NT - 1),
                )
            ctx_sb = att.tile([64, DE], f32, tag="ctx_sb", name="ctx_sb")
            nc.vector.tensor_scalar_mul(ctx_sb, ctx_ps, rinv)
            nc.sync.dma_start(out=ctx_d[r0 : r0 + 64, :], in_=ctx_sb)

        for s0 in range(8, 64, SUB):
            sub_batch(0, s0, SUB)
        sub_batch(1, 0, SUB)
        sub_batch(1, 8, SUB)
        epilogue(0)
        for s0 in range(16, 48, SUB):
            sub_batch(1, s0, SUB)
        sub_batch(1, 48, 8)
        sub_batch(1, 56, 4)
        sub_batch(1, 60, 4)
        epilogue(1)

    nc.compile()
    return nc


def _get_nc():
    if "nc" not in _CACHE:
        _CACHE["nc"] = _build_program()
    return _CACHE["nc"]


def _install_ntff_hook():
    """The agent image's antenv lacks axon_hooks; synthesize it so
    run_bass_kernel_spmd(trace=True) can reach the boot shim's
    ctypes-based NTFF profiler."""
    import sys
    import types

    if "antenv.axon_hooks" not in sys.modules:
        mod = types.ModuleType("antenv.axon_hooks")
        mod._hook = None
        mod.set_axon_ntff_profile_hook = lambda h: setattr(mod, "_hook", h)
        mod.get_axon_ntff_profile_hook = lambda: mod._hook
        sys.modules["antenv.axon_hooks"] = mod
        try:
            from trn_agent_boot.trn_boot import _ntff_profile_via_ctypes

            mod._hook = _ntff_profile_via_ctypes("/opt/axon/libaxon_pjrt.so")
        except Exception as e:
            print(f"ntff hook install failed: {e}")
    import concourse.bass_utils as bu

    bu.upload_artifacts = lambda tmpdir: "local://" + str(tmpdir)


def run(inputs, trace=False):
    from concourse.bass_utils import run_bass_kernel_spmd

    if trace:
        _install_ntff_hook()

    nc = _get_nc()
    enc = np.asarray(inputs["encoder_out"], dtype=np.float32)
    dec = np.asarray(inputs["decoder_out"], dtype=np.float32)
    w1 = np.ascontiguousarray(inputs["W1_w"], dtype=np.float32)
    w2 = np.ascontiguousarray(inputs["W2_w"], dtype=np.float32)
    v = np.ascontiguousarray(inputs["V_w"], dtype=np.float32)
    w1b = np.ascontiguousarray(inputs["W1_b"], dtype=np.float32)
    w2b = np.ascontiguousarray(inputs["W2_b"], dtype=np.float32)

    in_maps = []
    for c in range(N_CORES):
        b, h = c // 2, c % 2
        in_maps.append(
            {
                "enc": np.ascontiguousarray(enc[b]),
                "dec": np.ascontiguousarray(dec[b, h * ROWS : (h + 1) * ROWS]),
                "w1r": w1,
                "w2r": w2,
                "encr": np.ascontiguousarray(enc[b]),
                "v": v,
                "w1b": w1b,
                "w2b": w2b,
            }
        )

    res = run_bass_kernel_spmd(nc, in_maps, list(range(N_CORES)), trace=trace)

    context = np.empty((B, TD, DE), np.float32)
    attn = np.empty((B, TD, TE), np.float32)
    for c in range(N_CORES):
        b, h = c // 2, c % 2
        context[b, h * ROWS : (h + 1) * ROWS] = res.results[c]["ctx"]
        attn[b, h * ROWS : (h + 1) * ROWS] = res.results[c]["attn"]
    return (context, attn), res


def kernel(**inputs):
    (context, attn), _ = run(inputs)
    return context, attn


# revision 24
# speedup vs baseline: 1.0157x; 1.0124x over previous
"""Bahdanau additive attention on 8 Trainium2 NeuronCores.

Shapes (hardcoded from the problem spec):
  encoder_out [B=4, Te=512, De=512], decoder_out [B=4, Td=256, Dd=512]
  W1 [512,128], W2 [512,128], V [128,1]; U=128.
Outputs: context [4,256,512], attn_weights [4,256,512].

Sharding: core c handles batch b=c//2, decoder rows (c%2)*128..+128.

Per-core pipeline (U=128 on SBUF partitions for phase 1):
  encT    = PE-transpose(enc)                  [De,Te]   (per-te-chunk pipelined)
  enc_pT  = W1^T @ encT (+b1)                  [U,Te]    (fp32r single-pass)
  dec_pT  = W2^T @ decT (+b2)                  [U,Td]
  per td: pre = enc_pT + dec_pT[:,td]          (DVE/GPSIMD tensor_scalar_add)
          h   = tanh(pre) -> bf16              (ACT, batched over SUB tds)
  scores land in NATURAL [td, te] layout via accumulating matmuls with a
  sliding-window stationary (v in column j):
          score[32-group] += Zwin_j^T @ h_td_j   (PE, bf16 moving N=512)
  softmax row-wise: exp+accum_out (ACT) -> rinv (DVE) -> attn = esc*rinv
  ctx per 32-td group: PE-transpose attn quarter -> ctx = attnT^T @ enc (fp32r)
"""

import numpy as np

B, TE, TD, DE, U = 4, 512, 256, 512, 128
N_CORES = 8
ROWS = 128  # decoder rows per core
SUB = 8  # tds per tanh batch
GRP = 32  # tds per score accumulation group / ctx quarter

_CACHE = {}


def _build_program():
    from contextlib import ExitStack

    import concourse.bacc as bacc
    import concourse.tile as tile
    from concourse import mybir
    from concourse.masks import make_identity

    f32 = mybir.dt.float32
    f32r = mybir.dt.float32r
    bf16 = mybir.dt.bfloat16
    AF = mybir.ActivationFunctionType

    nc = bacc.Bacc("TRN2", target_bir_lowering=False, debug=False)

    enc_d = nc.dram_tensor("enc", [TE, DE], f32r, kind="ExternalInput")
    dec_d = nc.dram_tensor("dec", [ROWS, DE], f32r, kind="ExternalInput")
    w1r_d = nc.dram_tensor("w1r", [DE, U], f32r, kind="ExternalInput")
    w2r_d = nc.dram_tensor("w2r", [DE, U], f32r, kind="ExternalInput")
    v_d = nc.dram_tensor("v", [U, 1], f32, kind="ExternalInput")
    w1b_d = nc.dram_tensor("w1b", [U], f32, kind="ExternalInput")
    w2b_d = nc.dram_tensor("w2b", [U], f32, kind="ExternalInput")
    ctx_d = nc.dram_tensor("ctx", [ROWS, DE], f32, kind="ExternalOutput")
    attn_d = nc.dram_tensor("attn", [ROWS, TE], f32, kind="ExternalOutput")

    NT = TE // 128  # te chunks
    ND = DE // 128  # de chunks

    with tile.TileContext(nc) as tc, ExitStack() as ctx:
        const = ctx.enter_context(tc.tile_pool(name="const", bufs=1))
        work = ctx.enter_context(tc.tile_pool(name="work", bufs=3))
        att = ctx.enter_context(tc.tile_pool(name="att", bufs=2))
        ps_t = ctx.enter_context(tc.tile_pool(name="ps_t", bufs=2, space="PSUM"))
        ps_p = ctx.enter_context(tc.tile_pool(name="ps_p", bufs=1, space="PSUM"))
        ps_v = ctx.enter_context(tc.tile_pool(name="ps_v", bufs=2, space="PSUM"))
        ps_c = ctx.enter_context(tc.tile_pool(name="ps_c", bufs=2, space="PSUM"))

        ident = const.tile([128, 128], f32, tag="ident")
        make_identity(nc, ident)

        # --- input DMAs, split across the two HWDGE rings ---
        enc_sb = [
            const.tile([128, DE], f32r, tag=f"enc_{t}", name=f"enc_{t}")
            for t in range(NT)
        ]
        dec_sb = const.tile([ROWS, DE], f32r, tag="dec")
        ident_r = const.tile([128, 128], f32r, tag="ident_r")
        nc.vector.tensor_copy(ident_r, ident)
        # ring1 (SP): enc0, w2, w1, enc2 ...; ring2 (ACT): dec, enc1, enc3 ...
        nc.sync.dma_start(out=enc_sb[0], in_=enc_d[0:128, :])
        nc.scalar.dma_start(out=dec_sb, in_=dec_d[:, :])
        w2_r = const.tile([128, ND, U], f32r, tag="w2r")
        nc.sync.dma_start(out=w2_r, in_=w2r_d.rearrange("(k p) u -> p k u", p=128))
        w1_r = const.tile([128, ND, U], f32r, tag="w1r")
        nc.sync.dma_start(out=w1_r, in_=w1r_d.rearrange("(k p) u -> p k u", p=128))
        nc.scalar.dma_start(out=enc_sb[1], in_=enc_d[128:256, :])
        nc.sync.dma_start(out=enc_sb[2], in_=enc_d[256:384, :])
        nc.scalar.dma_start(out=enc_sb[3], in_=enc_d[384:512, :])
        v_sb = const.tile([U, 1], f32, tag="v")
        nc.sync.dma_start(out=v_sb, in_=v_d[:, :])
        w1b_sb = const.tile([U, 1], f32, tag="w1b")
        nc.sync.dma_start(out=w1b_sb, in_=w1b_d[:, None])
        w2b_sb = const.tile([U, 1], f32, tag="w2b")
        nc.scalar.dma_start(out=w2b_sb, in_=w2b_d[:, None])
        enc_r = enc_sb

        # sliding-window stationary: Zwin[:, (GRP-1)-j : (2*GRP-1)-j] puts
        # v (bf16) in column j of a [U, GRP] stationary, zeros elsewhere
        zwin = const.tile([U, 2 * GRP - 1], bf16, tag="zwin")
        nc.vector.memset(zwin, 0.0)
        nc.vector.tensor_copy(zwin[:, GRP - 1 : GRP], v_sb)

        # --- setup interleaved with early (te-chunked) tanh for tds 0..7 ---
        # encT stored d-major: encT_d [de-part, te] f32r
        encT = [
            const.tile([128, TE], f32r, tag=f"encT_{d}", name=f"encT_{d}")
            for d in range(ND)
        ]
        ep = ps_p.tile([U, TE], f32, tag="ep", name="ep")
        enc_pT = const.tile([U, TE], bf16, tag="enc_pT")
        attn_sb = const.tile([ROWS, TE], f32, tag="attn_sb")
        vout0 = ps_v.tile([64, TE], f32, tag="vout", name="vout0")
        pre_r = work.tile([128, 8, TE], bf16, tag="pre_r", bufs=1)
        th_r = work.tile([128, 8, TE], bf16, tag="th_r", bufs=1)

        dec_pT = None

        def enc_chunk(t):
            tp = ps_t.tile([128, ND, 128], f32r, tag="tp", name=f"tp_e{t}")
            for d in range(ND):
                nc.tensor.transpose(
                    tp[:, d, :], enc_sb[t][:, d * 128 : (d + 1) * 128], ident_r
                )
            for d in range(ND):
                nc.vector.tensor_copy(encT[d][:, t * 128 : (t + 1) * 128], tp[:, d, :])
            sl = slice(t * 128, (t + 1) * 128)
            for d in range(ND):
                nc.tensor.matmul(
                    ep[:, sl],
                    w1_r[:, d, :],
                    encT[d][:, sl],
                    start=(d == 0),
                    stop=(d == ND - 1),
                )
            nc.vector.tensor_scalar_add(enc_pT[:, sl], ep[:, sl], w1b_sb)

        def ramp_chunk(t):
            sl = slice(t * 128, (t + 1) * 128)
            for j in range(8):
                nc.vector.tensor_scalar_add(
                    pre_r[:, j, sl], enc_pT[:, sl], dec_pT[:, j : j + 1]
                )
            nc.scalar.activation(th_r[:, :, sl], pre_r[:, :, sl], AF.Tanh)

        enc_chunk(0)

        # dec: transpose + proj + bias, right after chunk 0
        tpd = ps_t.tile([128, ND, 128], f32r, tag="tp", name="tp_d")
        for d in range(ND):
            nc.tensor.transpose(tpd[:, d, :], dec_sb[:, d * 128 : (d + 1) * 128], ident_r)
        decT = const.tile([128, ND, 128], f32r, tag="decT")
        nc.vector.tensor_copy(decT, tpd)
        dp = ps_p.tile([U, ROWS], f32, tag="dp", name="dp")
        for d in range(ND):
            nc.tensor.matmul(
                dp,
                w2_r[:, d, :],
                decT[:, d, :],
                start=(d == 0),
                stop=(d == ND - 1),
            )
        dec_pT = const.tile([U, ROWS], f32, tag="dec_pT")
        nc.vector.tensor_scalar_add(dec_pT, dp, w2b_sb)

        ramp_chunk(0)
        for t in range(1, NT):
            enc_chunk(t)
            ramp_chunk(t)
        for j in range(8):
            nc.tensor.matmul(
                vout0[0:GRP, :],
                zwin[:, (GRP - 1) - j : (2 * GRP - 1) - j],
                th_r[:, j, :],
                start=(j == 0),
                stop=False,
            )

        # --- adds + tanh + score accumulation (tds 8..127), flat schedule ---
        # epilogue (softmax+ctx) for a half is emitted AFTER the next half's
        # first two sub-batches so DVE's in-order stream keeps feeding ACT.
        vout1 = ps_v.tile([64, TE], f32, tag="vout", name="vout1")
        vouts = [vout0, vout1]

        def sub_batch(half, s0, ns):
            vout = vouts[half]
            pre = work.tile([128, SUB, TE], bf16, tag="pre", name="pre")
            th = work.tile([128, SUB, TE], bf16, tag="th", name="th")
            for j in range(ns):
                td = half * 64 + s0 + j
                nc.vector.tensor_scalar_add(
                    pre[:, j, :], enc_pT, dec_pT[:, td : td + 1]
                )
            nc.scalar.activation(th[:, :ns, :], pre[:, :ns, :], AF.Tanh)
            for j in range(ns):
                r = s0 + j
                g = r // GRP
                jj = r % GRP
                nc.tensor.matmul(
                    vout[g * GRP : (g + 1) * GRP, :],
                    zwin[:, (GRP - 1) - jj : (2 * GRP - 1) - jj],
                    th[:, j, :],
                    start=(jj == 0),
                    stop=(jj == GRP - 1),
                )

        def epilogue(half):
            # softmax rows (no max subtraction: |score| <= |v|_1 ~ 9)
            r0 = half * 64
            vout = vouts[half]
            esc = att.tile([64, TE], f32, tag="esc", name="esc")
            esum = work.tile([64, 1], f32, tag="esum", name="esum")
            nc.scalar.activation(esc, vout, AF.Exp, accum_out=esum)
            rinv = work.tile([64, 1], f32, tag="rinv", name="rinv")
            nc.vector.reciprocal(rinv, esum)
            nc.vector.tensor_scalar_mul(attn_sb[r0 : r0 + 64, :], esc, rinv)
            nc.sync.dma_start(
                out=attn_d[r0 : r0 + 64, :], in_=attn_sb[r0 : r0 + 64, :]
            )
            # context from unnormalized esc; normalization fused in the copy
            at = ps_t.tile([128, NT, 64], f32, tag="tp", name=f"at{half}")
            for t in range(NT):
                nc.tensor.transpose(
                    at[:, t, :],
                    esc[:, t * 128 : (t + 1) * 128],
                    ident[0:64, 0:64],
                )
            escT = att.tile([128, NT, 64], f32r, tag="escT", name="escT")
            nc.vector.tensor_copy(escT, at)
            ctx_ps = ps_c.tile([64, DE], f32, tag="ctx", name="ctx_ps")
            for t in range(NT):
                nc.tensor.matmul(
                    ctx_ps,
                    escT[:, t, :],
                    enc_r[t],
                    start=(t == 0),
                    stop=(t == NT - 1),
                )
            ctx_sb = att.tile([64, DE], f32, tag="ctx_sb", name="ctx_sb")
            nc.vector.tensor_scalar_mul(ctx_sb, ctx_ps, rinv)
            nc.sync.dma_start(out=ctx_d[r0 : r0 + 64, :], in_=ctx_sb)

        for s0 in range(8, 64, SUB):
            sub_batch(0, s0, SUB)
        sub_batch(1, 0, SUB)
        sub_batch(1, 8, SUB)
        epilogue(0)
        for s0 in range(16, 48, SUB):
            sub_batch(1, s0, SUB)
        sub_batch(1, 48, 8)
        sub_batch(1, 56, 4)
        sub_batch(1, 60, 4)
        epilogue(1)

    nc.compile()
    return nc


def _get_nc():
    if "nc" not in _CACHE:
        _CACHE["nc"] = _build_program()
    return _CACHE["nc"]


def _install_ntff_hook():
    """The agent image's antenv lacks axon_hooks; synthesize it so
    run_bass_kernel_spmd(trace=True) can reach the boot shim's
    ctypes-based NTFF profiler."""
    import sys
    import types

    if "antenv.axon_hooks" not in sys.modules:
        mod = types.ModuleType("antenv.axon_hooks")
        mod._hook = None
        mod.set_axon_ntff_profile_hook = lambda h: setattr(mod, "_hook", h)
        mod.get_axon_ntff_profile_hook = lambda: mod._hook
        sys.modules["antenv.axon_hooks"] = mod
        try:
            from trn_agent_boot.trn_boot import _ntff_profile_via_ctypes

            mod._hook = _ntff_profile_via_ctypes("/opt/axon/libaxon_pjrt.so")
        except Exception as e:
            print(f"ntff hook install failed: {e}")
    import concourse.bass_utils as bu

    bu.upload_artifacts = lambda tmpdir: "local://" + str(tmpdir)


def run(inputs, trace=False):
    from concourse.bass_utils import run_bass_kernel_spmd

    if trace:
        _install_ntff_hook()

    nc = _get_nc()
    enc = np.asarray(inputs["encoder_out"], dtype=np.float32)
    dec = np.asarray(inputs["decoder_out"], dtype=np.float32)
    w1 = np.ascontiguousarray(inputs["W1_w"], dtype=np.float32)
    w2 = np.ascontiguousarray(inputs["W2_w"], dtype=np.float32)
    v = np.ascontiguousarray(inputs["V_w"], dtype=np.float32)
    w1b = np.ascontiguousarray(inputs["W1_b"], dtype=np.float32)
    w2b = np.ascontiguousarray(inputs["W2_b"], dtype=np.float32)

    in_maps = []
    for c in range(N_CORES):
        b, h = c // 2, c % 2
        in_maps.append(
            {
                "enc": np.ascontiguousarray(enc[b]),
                "dec": np.ascontiguousarray(dec[b, h * ROWS : (h + 1) * ROWS]),
                "w1r": w1,
                "w2r": w2,
                "v": v,
                "w1b": w1b,
                "w2b": w2b,
            }
        )

    res = run_bass_kernel_spmd(nc, in_maps, list(range(N_CORES)), trace=trace)

    context = np.empty((B, TD, DE), np.float32)
    attn = np.empty((B, TD, TE), np.float32)
    for c in range(N_CORES):
        b, h = c // 2, c % 2
        context[b, h * ROWS : (h + 1) * ROWS] = res.results[c]["ctx"]
        attn[b, h * ROWS : (h + 1) * ROWS] = res.results[c]["attn"]
    return (context, attn), res


def kernel(**inputs):
    (context, attn), _ = run(inputs)
    return context, attn
